# revision 46
# baseline (speedup 1.0000x reference)
"""Trainium2 Bass kernel for pre-LN multi-head self-attention.

Module: y = LN(x); qkv = y @ w_qkv; attention(8 heads, dh=64); out = ao @ w_out
Shapes: x [4, 2048, 512], w_qkv [512, 1536], w_out [512, 512], fp32.

Sharding (8 cores): core c -> batch b = c//2, head-group g = c%2 (4 heads).
Each core computes LN + QKV (its head slice) + attention + a partial output
projection (its heads' rows of w_out); the host sums the two partials per batch.

Per-core dataflow (transpose-free except one PE transpose of y):
  LN in natural [tok, d] layout (bn_stats) -> PE-transpose y -> yT [d, tok]
  Q^T, K^T = w^T @ yT   (features on partitions -- natural lhsT layout)
  V natural [tok, feat] with a fused ones-column so attn@V also accumulates
  the softmax denominator (row 64 of the PSUM accumulator).
  scoresT [k, q] = K^T.T @ Q^T per 128-k-token block.  The two heads of a
  head-pair sit at partitions 0-63 / 64-127 of qT/kT (PE row groups 0/64),
  so their K=64 score matmuls are emitted as a row-tiled pair that runs
  CONCURRENTLY on the array: one item = (512-q-block, head-pair, k-block),
  its two score halves landing in one [128,1024] PSUM tile, exp'd by a
  single ACT instruction.  attn@V accumulates per head over k-blocks in
  PSUM (two independent 1-bank accumulators); per-head 1/sumexp is
  broadcast across the 64 dh partitions with a K=1 PE matmul against a
  ones-row (no DRAM roundtrip), then reciprocal+mult on DVE; the output
  projection consumes aoT directly as lhsT.
ln_scale/ln_bias are folded into w_qkv on the host (w_eff = scale*W,
bias_row = bias@W added per-feature on device), so the device LN is pure
normalize.  Matmul operands are bf16 (PSUM accumulation stays fp32).
Stage D runs a depth-2 software pipeline with attn@V(i) issued BEFORE
scores(i+2) so the in-order PE never stalls on a score-PSUM bank held by
a pending exp; row-tiled scores + full-K attn@V keep the array active
enough for the HAM clock gate to hold K=8/8 (2.4 GHz).  Normalize work
for a finished unit is deferred two items so its PE broadcast never waits
on the DVE eviction copy.  A dummy exp after stage A hoists the ACT
exp-table load out of the attention phase.
"""

import sys

if "/opt/trn_rl_repo" not in sys.path:
    sys.path.insert(0, "/opt/trn_rl_repo")

from contextlib import ExitStack

import numpy as np

import concourse.bass as bass
import concourse.tile as tile
from concourse.masks import make_identity
from concourse import bacc, mybir
from concourse.bass_utils import run_bass_kernel_spmd

B, N, D = 4, 2048, 512
H, DH = 8, 64
HPC = 4                 # heads per core
FPC = HPC * DH          # 256 features per core
P = 128
NT = N // P             # 16 token tiles
DT = D // P             # 4 d tiles
NQ = N // 512           # 4 q-blocks of 512
EPS = 1e-6
SCALE = DH ** -0.5
F32 = mybir.dt.float32
F32R = mybir.dt.float32r
BF16 = mybir.dt.bfloat16
ALU = mybir.AluOpType
AFT = mybir.ActivationFunctionType




def build_kernel():
    nc = bacc.Bacc("TRN2", target_bir_lowering=False, debug=False)
    xb = nc.dram_tensor("xb", [N, D], F32, kind="ExternalInput").ap()
    wq = nc.dram_tensor("wq", [D, FPC], BF16, kind="ExternalInput").ap()
    wk = nc.dram_tensor("wk", [D, FPC], BF16, kind="ExternalInput").ap()
    wv = nc.dram_tensor("wv", [D, FPC], BF16, kind="ExternalInput").ap()
    wo = nc.dram_tensor("wo", [FPC, D], BF16, kind="ExternalInput").ap()
    bq = nc.dram_tensor("bq", [FPC], F32, kind="ExternalInput").ap()
    bk = nc.dram_tensor("bk", [FPC], F32, kind="ExternalInput").ap()
    bv = nc.dram_tensor("bv", [FPC], F32, kind="ExternalInput").ap()
    out = nc.dram_tensor("out", [N, D], F32, kind="ExternalOutput").ap()

    with tile.TileContext(nc, pool_alloc_mode="queue") as tc, ExitStack() as ctx:
        consts = ctx.enter_context(tc.tile_pool(name="consts", bufs=1))

        big = ctx.enter_context(tc.tile_pool(name="big", bufs=1))

        identity = consts.tile([P, P], BF16)
        make_identity(nc, identity)
        eps_t = consts.tile([P, 1], F32)
        nc.vector.memset(eps_t, EPS)
        # ones row living at PARTITION 64: lhsT for the K=1 denominator
        # broadcast must share its base partition with the rhs (the denom row
        # sits at partition 64 of the attn@V accumulator)
        ones_m = consts.tile([DH + 1, DH], BF16)
        nc.vector.memset(ones_m, 1.0)
        ones_row = ones_m[DH : DH + 1, :]
        exp_warm = consts.tile([P, 1], F32)
        warm_sb = consts.tile([P, 512], BF16)
        nc.vector.memset(warm_sb, 0.0)

        yT = [big.tile([P, N], BF16, tag=f"yT{j}", name=f"yT{j}") for j in range(DT)]
        qT = [big.tile([P, N], BF16, tag=f"qT{j}", name=f"qT{j}") for j in range(2)]
        kT = [big.tile([P, N], BF16, tag=f"kT{j}", name=f"kT{j}") for j in range(2)]
        aoT = [big.tile([P, N], BF16, tag=f"aoT{j}", name=f"aoT{j}") for j in range(2)]
        v_sb = big.tile([P, NT, HPC, DH + 1], BF16)
        ones_col = consts.tile([P, 1], F32)
        nc.vector.memset(ones_col, 1.0)
        nc.vector.tensor_copy(
            v_sb[:, :, :, DH : DH + 1],
            ones_col[:, 0:1].to_broadcast((P, NT, HPC, 1)),
        )

        # weights: [d, f] -> sbuf [p, dt, f] -- issued before the LN phase so
        # the transfers overlap it and QKV chunks can start with token-group 0
        w_k_sb = consts.tile([P, DT, FPC], BF16)
        nc.sync.dma_start(out=w_k_sb, in_=wk.rearrange("(t p) f -> p t f", p=P))
        w_q_sb = consts.tile([P, DT, FPC], BF16)
        nc.sync.dma_start(out=w_q_sb, in_=wq.rearrange("(t p) f -> p t f", p=P))
        w_v_sb = consts.tile([P, DT, FPC], BF16)
        nc.sync.dma_start(out=w_v_sb, in_=wv.rearrange("(t p) f -> p t f", p=P))
        w_o_sb = consts.tile([P, 2, D], BF16)
        nc.sync.dma_start(out=w_o_sb, in_=wo.rearrange("(t p) f -> p t f", p=P))
        bq_sb = consts.tile([P, 2], F32)
        nc.sync.dma_start(out=bq_sb, in_=bq.rearrange("(t p) -> p t", p=P))
        bk_sb = consts.tile([P, 2], F32)
        nc.sync.dma_start(out=bk_sb, in_=bk.rearrange("(t p) -> p t", p=P))
        bv_b = consts.tile([P, FPC], F32)
        bv_bcast = bass.AP(tensor=bv.tensor, offset=bv.offset, ap=[[0, P]] + list(bv.ap))
        nc.sync.dma_start(out=bv_b, in_=bv_bcast)

        # ---- Stages A-C interleaved per 4-tile token group: LayerNorm,
        # transpose y -> yT, then the QKV chunks for just that group's token
        # columns.  The PE's QKV matmuls overlap the next group's LN (DVE) ----
        with tc.tile_pool(name="ln", bufs=3) as ln, tc.tile_pool(
            name="tp_psum", bufs=2, space="PSUM"
        ) as tpp, tc.tile_pool(
            name="c_psum", bufs=2, space="PSUM"
        ) as cpp, tc.tile_pool(name="v_psum", bufs=2, space="PSUM") as vpp:
            # full-width dummy matmuls fill the PE-idle LayerNorm window: the
            # HAM clock gate un-throttles ~10us in, so the real transposes and
            # QKV chunks run at 2.4 GHz instead of warming up mid-prelude.
            # 49 covers until group-0's transposes begin (~18.5us): transposes
            # do not register as PE-busy (run12 re-throttled during them), so
            # the dummy stream must bridge the whole pre-transpose window
            wp = tpp.tile([P, 512], F32, tag="warm", bufs=1)
            for _ in range(49):
                nc.tensor.matmul(wp, lhsT=identity, rhs=warm_sb, start=True, stop=True)
            for ig in range(NT // 4):  # groups of 4 token tiles
                y_ts = []
                for ii in range(4):
                    i = ig * 4 + ii
                    x_t = ln.tile([P, D], F32, tag="x", bufs=5)
                    nc.sync.dma_start(out=x_t, in_=xb[i * P : (i + 1) * P, :])
                    stats = ln.tile([P, 6], F32, tag="stats")
                    nc.vector.bn_stats(out=stats, in_=x_t)
                    mv = ln.tile([P, 2], F32, tag="mv")
                    nc.vector.bn_aggr(out=mv, in_=stats)
                    std = ln.tile([P, 1], F32, tag="std")
                    nc.scalar.activation(
                        out=std, in_=mv[:, 1:2], func=AFT.Sqrt, bias=eps_t[:, 0:1]
                    )
                    rstd = ln.tile([P, 1], F32, tag="rstd")
                    nc.vector.reciprocal(out=rstd, in_=std)
                    if i == NT - 1:
                        # depends on the final Sqrt: forces the ACT exp table
                        # set to load AFTER the sqrt set, so it is resident
                        # for stage D (the scheduler reorders by deps, not
                        # program order)
                        nc.scalar.activation(out=exp_warm, in_=std, func=AFT.Exp)
                    y_t = ln.tile([P, D], BF16, tag="y", bufs=6)
                    nc.vector.tensor_scalar(
                        out=y_t,
                        in0=x_t,
                        scalar1=mv[:, 0:1],
                        scalar2=rstd[:, 0:1],
                        op0=ALU.subtract,
                        op1=ALU.mult,
                    )
                    y_ts.append(y_t)
                for j in range(DT):
                    pt = tpp.tile([P, 512], BF16, tag="tp")
                    for ii in range(4):
                        nc.tensor.transpose(
                            pt[:, ii * P : (ii + 1) * P],
                            y_ts[ii][:, j * P : (j + 1) * P],
                            identity,
                        )
                    nc.scalar.activation(
                        out=yT[j][:, ig * 512 : (ig + 1) * 512],
                        in_=pt,
                        func=AFT.Copy,
                    )
                if ig == 0:
                    # bridge the group-0 transpose window (transpose-mode does
                    # not register as PE-busy for the HAM activity monitor)
                    for _ in range(6):
                        nc.tensor.matmul(
                            wp, lhsT=identity, rhs=warm_sb, start=True, stop=True
                        )
                g0 = ig * 512
                for wi, (w_sb, b_sb, dstT) in enumerate(
                    ((w_k_sb, bk_sb, kT), (w_q_sb, bq_sb, qT))
                ):
                    for j in range(2):
                        ps = cpp.tile([P, 512], F32, tag="qk", name=f"qk{wi}{j}_{ig}")
                        for dt in range(DT):
                            nc.tensor.matmul(
                                ps,
                                lhsT=(w_sb[:, dt, j * P : (j + 1) * P]),
                                rhs=(yT[dt][:, g0 : g0 + 512]),
                                start=(dt == 0),
                                stop=(dt == DT - 1),
                            )
                        # bias-add evictions alternate ACT/DVE to balance the
                        # two psum-capable engines
                        if (wi + j) % 2 == 0:
                            nc.scalar.activation(
                                out=dstT[j][:, g0 : g0 + 512],
                                in_=ps,
                                func=AFT.Identity,
                                bias=b_sb[:, j : j + 1],
                            )
                        else:
                            nc.vector.tensor_scalar(
                                out=dstT[j][:, g0 : g0 + 512],
                                in0=ps,
                                scalar1=b_sb[:, j : j + 1],
                                scalar2=None,
                                op0=ALU.add,
                            )
                for i in range(ig * 4, ig * 4 + 4):
                    ps = vpp.tile([P, FPC], F32, tag="v", name=f"v{i}")
                    for dt in range(DT):
                        nc.tensor.matmul(
                            ps,
                            lhsT=(yT[dt][:, i * P : (i + 1) * P]),
                            rhs=(w_v_sb[:, dt, :]),
                            start=(dt == 0),
                            stop=(dt == DT - 1),
                        )
                    nc.vector.tensor_tensor(
                        out=v_sb[:, i, :, 0:DH],
                        in0=ps.rearrange("p (h d) -> p h d", h=HPC),
                        in1=bv_b.rearrange("p (h d) -> p h d", h=HPC),
                        op=ALU.add,
                    )

        # ---- Stage D: attention, units of (512-q-block, head-pair) ----
        # The two heads of a pair live at partitions 0-63 / 64-127 of qT/kT,
        # i.e. PE row groups 0 and 64: their K=64 score matmuls are emitted as
        # a row-tiled pair (tile_position (0,0) / (64,0)) and run CONCURRENTLY
        # on the array -- full 128-row activity per score step, 2x throughput,
        # and dense enough for the HAM clock gate to hold K=8/8.
        with tc.tile_pool(name="sc_psum", bufs=2, space="PSUM") as scp, tc.tile_pool(
            name="ao_psum", bufs=3, space="PSUM"
        ) as aop, tc.tile_pool(
            name="o_psum", bufs=1, space="PSUM"
        ) as opp, tc.tile_pool(name="exp_sb", bufs=6) as exps, tc.tile_pool(
            name="nrm", bufs=4
        ) as nrm, tc.tile_pool(name="o_sb", bufs=3) as osb:
            items = [
                (qb, pj, kb) for qb in range(NQ) for pj in range(2) for kb in range(NT)
            ]
            ex_tiles = {}
            ao_tiles = {}
            pending = []  # (ready_at_item, emit_closure) for unit normalizes

            def sc_exp(i):
                qb, pj, kb = items[i]
                q0 = qb * 512
                sc = scp.tile([P, 1024], F32, tag="sc", name=f"sc{i}")
                for c in range(2):
                    po = c * DH
                    nc.tensor.matmul(
                        sc[:, c * 512 : (c + 1) * 512],
                        lhsT=(kT[pj][po : po + DH, kb * P : (kb + 1) * P]),
                        rhs=(qT[pj][po : po + DH, q0 : q0 + 512]),
                        start=True,
                        stop=True,
                    )
                ex = exps.tile([P, 1024], BF16, tag="ex", name=f"ex{i}")
                nc.scalar.activation(out=ex, in_=sc, func=AFT.Exp, scale=SCALE)
                ex_tiles[i] = ex

            def make_norm(j, po, cs, ao_sb, uid):
                def norm():
                    # broadcast the denominator row across the dh partitions
                    # with a K=1 matmul (PSUM tile shared with the outproj
                    # ring -- usage windows never overlap), then normalize
                    bc = opp.tile([P, D], F32, tag="o", name=f"bc{uid}")
                    nc.tensor.matmul(
                        bc[0:DH, :],
                        lhsT=ones_row,
                        rhs=ao_sb[DH : DH + 1, :],
                        start=True,
                        stop=True,
                    )
                    rb = nrm.tile([DH, 512], F32, tag="rb", bufs=2, name=f"rb{uid}")
                    # ~51-ULP approx is ample for softmax denominators and ~5x
                    # faster than the exact multi-pass InstReciprocal
                    nc.vector.reciprocal_approx_fast(out=rb, in_=bc[0:DH, :])
                    nc.vector.tensor_tensor(
                        out=aoT[j][po : po + DH, cs : cs + 512],
                        in0=ao_sb[0:DH, :],
                        in1=rb,
                        op=ALU.mult,
                    )

                return norm

            def attn_v(i):
                qb, pj, kb = items[i]
                q0 = qb * 512
                if kb == 0:
                    ao_tiles[(qb, pj)] = (
                        aop.tile([DH + 1, 512], F32, tag="ao", name=f"aoA{qb}_{pj}"),
                        aop.tile([DH + 1, 512], F32, tag="ao", name=f"aoB{qb}_{pj}"),
                    )
                halves = ao_tiles[(qb, pj)]
                ex = ex_tiles.pop(i)
                for c in range(2):
                    nc.tensor.matmul(
                        halves[c],
                        lhsT=(v_sb[:, kb, 2 * pj + c, :]),
                        rhs=(ex[:, c * 512 : (c + 1) * 512]),
                        start=(kb == 0),
                        stop=(kb == NT - 1),
                    )
                if kb == NT - 1:
                    # evict the unnormalized accumulator halves; normalize is
                    # deferred so the PE broadcast never waits on these copies
                    for c in range(2):
                        # bf16 eviction: aoT is bf16 anyway, and the bf16
                        # denominator row makes the broadcast matmul stream at
                        # full rate (~240ns vs 570ns fp32)
                        ao_sb = nrm.tile(
                            [DH + 1, 512], BF16, tag="ao_sb", name=f"aosb{i}_{c}"
                        )
                        nc.vector.tensor_copy(ao_sb, halves[c])
                        # spread the two PE broadcasts well apart: each eats
                        # ~500ns of PE time and the steady-state slack is only
                        # ~100ns/item -- bunched they stall the exp stream
                        pending.append(
                            (i + 3 + 3 * c, make_norm(pj, c * DH, q0, ao_sb, f"{i}_{c}"))
                        )

            def outproj_tile(mt, pool, tag, evict_act=False):
                ps = pool.tile([P, D], F32, tag=tag, name=f"o{mt}")
                for kt in range(2):
                    nc.tensor.matmul(
                        ps,
                        lhsT=(aoT[kt][:, mt * P : (mt + 1) * P]),
                        rhs=(w_o_sb[:, kt, :]),
                        start=(kt == 0),
                        stop=(kt == 1),
                    )
                ot = osb.tile([P, D], F32, tag="ot", name=f"ot{mt}")
                if evict_act:
                    # tail only: ACT is idle there, DVE is busy with norms
                    nc.scalar.activation(out=ot, in_=ps, func=AFT.Copy)
                else:
                    nc.vector.tensor_copy(ot, ps)
                nc.sync.dma_start(out=out[mt * P : (mt + 1) * P, :], in_=ot)

            # q-block qb (tokens 512qb..512qb+511 = mt 4qb..4qb+3) is fully
            # normalized ~3 items after its second unit ends at 32qb+31;
            # spread its output-projection tiles one per item from 32qb+36
            # q-block qb is fully normalized once (qb,P1)'s R-norm lands at
            # item 32qb+37; outprojs start at +38 (emitting one EARLIER would
            # wedge the in-order PE behind a norm that is emitted later)
            inject = {}
            for qb in range(NQ - 1):
                for k in range(4):
                    inject[32 * qb + 38 + 3 * k] = 4 * qb + k

            DEPTH = 2
            for i in range(min(DEPTH, len(items))):
                sc_exp(i)
            for i in range(len(items)):
                attn_v(i)
                if i + DEPTH < len(items):
                    sc_exp(i + DEPTH)
                if pending and pending[0][0] <= i:
                    pending.pop(0)[1]()
                if i in inject:
                    outproj_tile(inject[i], opp, "o")
            for _, fn in pending:
                fn()
            # keep the PE array (and HAM clock) warm while DVE finishes the
            # last unit's normalize chain, so the final output projections run
            # at 2.4 GHz instead of re-throttled 1.2
            wt = scp.tile([P, 1024], F32, tag="sc", name="wtail")
            for _ in range(12):
                nc.tensor.matmul(
                    wt[:, 0:512], lhsT=identity, rhs=warm_sb, start=True, stop=True
                )
            for mt in range(4 * (NQ - 1), NT):
                outproj_tile(mt, opp, "o", evict_act=True)

    nc.compile()
    return nc


_NC_CACHE = None
_LAST_RESULT = None


def kernel(x, ln_scale, ln_bias, w_qkv, w_out):
    global _NC_CACHE, _LAST_RESULT
    if _NC_CACHE is None:
        _NC_CACHE = build_kernel()
    nc = _NC_CACHE

    import ml_dtypes

    x = np.asarray(x, np.float32)
    w_eff = (np.asarray(ln_scale, np.float32)[:, None] * np.asarray(w_qkv, np.float32))
    b_row = np.asarray(ln_bias, np.float32) @ np.asarray(w_qkv, np.float32)
    w_eff = w_eff.astype(ml_dtypes.bfloat16)
    w_out = np.asarray(w_out, np.float32).astype(ml_dtypes.bfloat16)

    in_maps = []
    for c in range(8):
        b, g = c // 2, c % 2
        s = slice(FPC * g, FPC * g + FPC)
        ks = slice(512 + FPC * g, 512 + FPC * g + FPC)
        vs = slice(1024 + FPC * g, 1024 + FPC * g + FPC)
        in_maps.append(
            {
                "xb": np.ascontiguousarray(x[b]),
                "wq": np.ascontiguousarray(w_eff[:, s]),
                "wk": np.ascontiguousarray(w_eff[:, ks]),
                "wv": np.ascontiguousarray(w_eff[:, vs]),
                "wo": np.ascontiguousarray(w_out[s, :]),
                "bq": np.ascontiguousarray(b_row[s]),
                "bk": np.ascontiguousarray(b_row[ks]),
                "bv": np.ascontiguousarray(b_row[vs]),
            }
        )
    res = run_bass_kernel_spmd(nc, in_maps, core_ids=list(range(8)))
    _LAST_RESULT = res
    outs = [res.results[c]["out"] for c in range(8)]
    return np.stack([outs[2 * b] + outs[2 * b + 1] for b in range(B)]).astype(
        np.float32
    )


if __name__ == "__main__":
    xs = np.random.randn(B, N, D).astype(np.float32)
    o = kernel(
        x=xs,
        ln_scale=np.ones(D, np.float32),
        ln_bias=np.zeros(D, np.float32),
        w_qkv=(np.random.randn(D, 3 * H * DH) / np.sqrt(D)).astype(np.float32),
        w_out=(np.random.randn(H * DH, D) / np.sqrt(H * DH)).astype(np.float32),
    )
    print(o.shape, o.dtype)


# revision 49
# speedup vs baseline: 1.0079x; 1.0079x over previous
"""Trainium2 Bass kernel for pre-LN multi-head self-attention.

Module: y = LN(x); qkv = y @ w_qkv; attention(8 heads, dh=64); out = ao @ w_out
Shapes: x [4, 2048, 512], w_qkv [512, 1536], w_out [512, 512], fp32.

Sharding (8 cores): core c -> batch b = c//2, head-group g = c%2 (4 heads).
Each core computes LN + QKV (its head slice) + attention + a partial output
projection (its heads' rows of w_out); the host sums the two partials per batch.

Per-core dataflow (transpose-free except one PE transpose of y):
  LN in natural [tok, d] layout (bn_stats) -> PE-transpose y -> yT [d, tok]
  Q^T, K^T = w^T @ yT   (features on partitions -- natural lhsT layout)
  V natural [tok, feat] with a fused ones-column so attn@V also accumulates
  the softmax denominator (row 64 of the PSUM accumulator).
  scoresT [k, q] = K^T.T @ Q^T per 128-k-token block.  The two heads of a
  head-pair sit at partitions 0-63 / 64-127 of qT/kT (PE row groups 0/64),
  so their K=64 score matmuls are emitted as a row-tiled pair that runs
  CONCURRENTLY on the array: one item = (512-q-block, head-pair, k-block),
  its two score halves landing in one [128,1024] PSUM tile, exp'd by a
  single ACT instruction.  attn@V accumulates per head over k-blocks in
  PSUM (two independent 1-bank accumulators); per-head 1/sumexp is
  broadcast across the 64 dh partitions with a K=1 PE matmul against a
  ones-row (no DRAM roundtrip), then reciprocal+mult on DVE; the output
  projection consumes aoT directly as lhsT.
ln_scale/ln_bias are folded into w_qkv on the host (w_eff = scale*W,
bias_row = bias@W added per-feature on device), so the device LN is pure
normalize.  Matmul operands are bf16 (PSUM accumulation stays fp32).
Stage D runs a depth-2 software pipeline with attn@V(i) issued BEFORE
scores(i+2) so the in-order PE never stalls on a score-PSUM bank held by
a pending exp; row-tiled scores + full-K attn@V keep the array active
enough for the HAM clock gate to hold K=8/8 (2.4 GHz).  Normalize work
for a finished unit is deferred two items so its PE broadcast never waits
on the DVE eviction copy.  A dummy exp after stage A hoists the ACT
exp-table load out of the attention phase.
"""

import sys

if "/opt/trn_rl_repo" not in sys.path:
    sys.path.insert(0, "/opt/trn_rl_repo")

from contextlib import ExitStack

import numpy as np

import concourse.bass as bass
import concourse.tile as tile
from concourse.masks import make_identity
from concourse import bacc, mybir
from concourse.bass_utils import run_bass_kernel_spmd

B, N, D = 4, 2048, 512
H, DH = 8, 64
HPC = 4                 # heads per core
FPC = HPC * DH          # 256 features per core
P = 128
NT = N // P             # 16 token tiles
DT = D // P             # 4 d tiles
NQ = N // 512           # 4 q-blocks of 512
EPS = 1e-6
SCALE = DH ** -0.5
F32 = mybir.dt.float32
F32R = mybir.dt.float32r
BF16 = mybir.dt.bfloat16
ALU = mybir.AluOpType
AFT = mybir.ActivationFunctionType




def build_kernel():
    nc = bacc.Bacc("TRN2", target_bir_lowering=False, debug=False)
    xb = nc.dram_tensor("xb", [N, D], F32, kind="ExternalInput").ap()
    wq = nc.dram_tensor("wq", [D, FPC], BF16, kind="ExternalInput").ap()
    wk = nc.dram_tensor("wk", [D, FPC], BF16, kind="ExternalInput").ap()
    wv = nc.dram_tensor("wv", [D, FPC], BF16, kind="ExternalInput").ap()
    wo = nc.dram_tensor("wo", [FPC, D], BF16, kind="ExternalInput").ap()
    bq = nc.dram_tensor("bq", [FPC], F32, kind="ExternalInput").ap()
    bk = nc.dram_tensor("bk", [FPC], F32, kind="ExternalInput").ap()
    bv = nc.dram_tensor("bv", [FPC], F32, kind="ExternalInput").ap()
    out = nc.dram_tensor("out", [N, D], F32, kind="ExternalOutput").ap()

    with tile.TileContext(nc, pool_alloc_mode="queue") as tc, ExitStack() as ctx:
        consts = ctx.enter_context(tc.tile_pool(name="consts", bufs=1))

        big = ctx.enter_context(tc.tile_pool(name="big", bufs=1))

        identity = consts.tile([P, P], BF16)
        make_identity(nc, identity)
        eps_t = consts.tile([P, 1], F32)
        nc.vector.memset(eps_t, EPS)
        # ones row living at PARTITION 64: lhsT for the K=1 denominator
        # broadcast must share its base partition with the rhs (the denom row
        # sits at partition 64 of the attn@V accumulator)
        ones_m = consts.tile([DH + 1, DH], BF16)
        nc.vector.memset(ones_m, 1.0)
        ones_row = ones_m[DH : DH + 1, :]
        exp_warm = consts.tile([P, 1], F32)
        warm_sb = consts.tile([P, 512], BF16)
        nc.vector.memset(warm_sb, 0.0)

        yT = [big.tile([P, N], BF16, tag=f"yT{j}", name=f"yT{j}") for j in range(DT)]
        qT = [big.tile([P, N], BF16, tag=f"qT{j}", name=f"qT{j}") for j in range(2)]
        kT = [big.tile([P, N], BF16, tag=f"kT{j}", name=f"kT{j}") for j in range(2)]
        aoT = [big.tile([P, N], BF16, tag=f"aoT{j}", name=f"aoT{j}") for j in range(2)]
        v_sb = big.tile([P, NT, HPC, DH + 1], BF16)
        ones_col = consts.tile([P, 1], F32)
        nc.vector.memset(ones_col, 1.0)
        nc.vector.tensor_copy(
            v_sb[:, :, :, DH : DH + 1],
            ones_col[:, 0:1].to_broadcast((P, NT, HPC, 1)),
        )

        # weights: [d, f] -> sbuf [p, dt, f] -- issued before the LN phase so
        # the transfers overlap it and QKV chunks can start with token-group 0
        w_k_sb = consts.tile([P, DT, FPC], BF16)
        nc.sync.dma_start(out=w_k_sb, in_=wk.rearrange("(t p) f -> p t f", p=P))
        w_q_sb = consts.tile([P, DT, FPC], BF16)
        nc.sync.dma_start(out=w_q_sb, in_=wq.rearrange("(t p) f -> p t f", p=P))
        w_v_sb = consts.tile([P, DT, FPC], BF16)
        nc.sync.dma_start(out=w_v_sb, in_=wv.rearrange("(t p) f -> p t f", p=P))
        w_o_sb = consts.tile([P, 2, D], BF16)
        nc.sync.dma_start(out=w_o_sb, in_=wo.rearrange("(t p) f -> p t f", p=P))
        bq_sb = consts.tile([P, 2], F32)
        nc.sync.dma_start(out=bq_sb, in_=bq.rearrange("(t p) -> p t", p=P))
        bk_sb = consts.tile([P, 2], F32)
        nc.sync.dma_start(out=bk_sb, in_=bk.rearrange("(t p) -> p t", p=P))
        bv_b = consts.tile([P, FPC], F32)
        bv_bcast = bass.AP(tensor=bv.tensor, offset=bv.offset, ap=[[0, P]] + list(bv.ap))
        nc.sync.dma_start(out=bv_b, in_=bv_bcast)

        # ---- Stages A-C interleaved per 4-tile token group: LayerNorm,
        # transpose y -> yT, then the QKV chunks for just that group's token
        # columns.  The PE's QKV matmuls overlap the next group's LN (DVE) ----
        with tc.tile_pool(name="ln", bufs=3) as ln, tc.tile_pool(
            name="tp_psum", bufs=2, space="PSUM"
        ) as tpp, tc.tile_pool(
            name="c_psum", bufs=2, space="PSUM"
        ) as cpp, tc.tile_pool(name="v_psum", bufs=2, space="PSUM") as vpp:
            # full-width dummy matmuls fill the PE-idle LayerNorm window: the
            # HAM clock gate un-throttles ~10us in, so the real transposes and
            # QKV chunks run at 2.4 GHz instead of warming up mid-prelude.
            # 36 covers the window until group-0's transposes (~18us) -- any
            # >3.4us PE-idle stretch re-throttles the clock
            wp = tpp.tile([P, 512], F32, tag="warm", bufs=1)
            for _ in range(36):
                nc.tensor.matmul(wp, lhsT=identity, rhs=warm_sb, start=True, stop=True)
            for ig in range(NT // 4):  # groups of 4 token tiles
                y_ts = []
                for ii in range(4):
                    i = ig * 4 + ii
                    x_t = ln.tile([P, D], F32, tag="x", bufs=5)
                    nc.sync.dma_start(out=x_t, in_=xb[i * P : (i + 1) * P, :])
                    stats = ln.tile([P, 6], F32, tag="stats")
                    nc.vector.bn_stats(out=stats, in_=x_t)
                    mv = ln.tile([P, 2], F32, tag="mv")
                    nc.vector.bn_aggr(out=mv, in_=stats)
                    std = ln.tile([P, 1], F32, tag="std")
                    nc.scalar.activation(
                        out=std, in_=mv[:, 1:2], func=AFT.Sqrt, bias=eps_t[:, 0:1]
                    )
                    rstd = ln.tile([P, 1], F32, tag="rstd")
                    nc.vector.reciprocal(out=rstd, in_=std)
                    if i == NT - 1:
                        # depends on the final Sqrt: forces the ACT exp table
                        # set to load AFTER the sqrt set, so it is resident
                        # for stage D (the scheduler reorders by deps, not
                        # program order)
                        nc.scalar.activation(out=exp_warm, in_=std, func=AFT.Exp)
                    y_t = ln.tile([P, D], BF16, tag="y", bufs=6)
                    nc.vector.tensor_scalar(
                        out=y_t,
                        in0=x_t,
                        scalar1=mv[:, 0:1],
                        scalar2=rstd[:, 0:1],
                        op0=ALU.subtract,
                        op1=ALU.mult,
                    )
                    y_ts.append(y_t)
                for j in range(DT):
                    pt = tpp.tile([P, 512], BF16, tag="tp")
                    for ii in range(4):
                        nc.tensor.transpose(
                            pt[:, ii * P : (ii + 1) * P],
                            y_ts[ii][:, j * P : (j + 1) * P],
                            identity,
                        )
                    nc.scalar.activation(
                        out=yT[j][:, ig * 512 : (ig + 1) * 512],
                        in_=pt,
                        func=AFT.Copy,
                    )
                if ig == 0:
                    # bridge the group-0 transpose window (transpose-mode does
                    # not register as PE-busy for the HAM activity monitor)
                    for _ in range(6):
                        nc.tensor.matmul(
                            wp, lhsT=identity, rhs=warm_sb, start=True, stop=True
                        )
                g0 = ig * 512
                for wi, (w_sb, b_sb, dstT) in enumerate(
                    ((w_k_sb, bk_sb, kT), (w_q_sb, bq_sb, qT))
                ):
                    for j in range(2):
                        ps = cpp.tile([P, 512], F32, tag="qk", name=f"qk{wi}{j}_{ig}")
                        for dt in range(DT):
                            nc.tensor.matmul(
                                ps,
                                lhsT=(w_sb[:, dt, j * P : (j + 1) * P]),
                                rhs=(yT[dt][:, g0 : g0 + 512]),
                                start=(dt == 0),
                                stop=(dt == DT - 1),
                            )
                        # bias-add evictions alternate ACT/DVE to balance the
                        # two psum-capable engines
                        if (wi + j) % 2 == 0:
                            nc.scalar.activation(
                                out=dstT[j][:, g0 : g0 + 512],
                                in_=ps,
                                func=AFT.Identity,
                                bias=b_sb[:, j : j + 1],
                            )
                        else:
                            nc.vector.tensor_scalar(
                                out=dstT[j][:, g0 : g0 + 512],
                                in0=ps,
                                scalar1=b_sb[:, j : j + 1],
                                scalar2=None,
                                op0=ALU.add,
                            )
                for i in range(ig * 4, ig * 4 + 4):
                    ps = vpp.tile([P, FPC], F32, tag="v", name=f"v{i}")
                    for dt in range(DT):
                        nc.tensor.matmul(
                            ps,
                            lhsT=(yT[dt][:, i * P : (i + 1) * P]),
                            rhs=(w_v_sb[:, dt, :]),
                            start=(dt == 0),
                            stop=(dt == DT - 1),
                        )
                    nc.vector.tensor_tensor(
                        out=v_sb[:, i, :, 0:DH],
                        in0=ps.rearrange("p (h d) -> p h d", h=HPC),
                        in1=bv_b.rearrange("p (h d) -> p h d", h=HPC),
                        op=ALU.add,
                    )

        # ---- Stage D: attention, units of (512-q-block, head-pair) ----
        # The two heads of a pair live at partitions 0-63 / 64-127 of qT/kT,
        # i.e. PE row groups 0 and 64: their K=64 score matmuls are emitted as
        # a row-tiled pair (tile_position (0,0) / (64,0)) and run CONCURRENTLY
        # on the array -- full 128-row activity per score step, 2x throughput,
        # and dense enough for the HAM clock gate to hold K=8/8.
        with tc.tile_pool(name="sc_psum", bufs=2, space="PSUM") as scp, tc.tile_pool(
            name="ao_psum", bufs=3, space="PSUM"
        ) as aop, tc.tile_pool(
            name="o_psum", bufs=1, space="PSUM"
        ) as opp, tc.tile_pool(name="exp_sb", bufs=6) as exps, tc.tile_pool(
            name="nrm", bufs=4
        ) as nrm, tc.tile_pool(name="o_sb", bufs=3) as osb:
            items = [
                (qb, pj, kb) for qb in range(NQ) for pj in range(2) for kb in range(NT)
            ]
            ex_tiles = {}
            ao_tiles = {}
            pending = []  # (ready_at_item, emit_closure) for unit normalizes

            def sc_exp(i):
                qb, pj, kb = items[i]
                q0 = qb * 512
                sc = scp.tile([P, 1024], F32, tag="sc", name=f"sc{i}")
                for c in range(2):
                    po = c * DH
                    nc.tensor.matmul(
                        sc[:, c * 512 : (c + 1) * 512],
                        lhsT=(kT[pj][po : po + DH, kb * P : (kb + 1) * P]),
                        rhs=(qT[pj][po : po + DH, q0 : q0 + 512]),
                        start=True,
                        stop=True,
                    )
                ex = exps.tile([P, 1024], BF16, tag="ex", name=f"ex{i}")
                nc.scalar.activation(out=ex, in_=sc, func=AFT.Exp, scale=SCALE)
                ex_tiles[i] = ex

            def make_norm(j, po, cs, ao_sb, uid):
                def norm():
                    # broadcast the denominator row across the dh partitions
                    # with a K=1 matmul (PSUM tile shared with the outproj
                    # ring -- usage windows never overlap), then normalize
                    bc = opp.tile([P, D], F32, tag="o", name=f"bc{uid}")
                    nc.tensor.matmul(
                        bc[0:DH, :],
                        lhsT=ones_row,
                        rhs=ao_sb[DH : DH + 1, :],
                        start=True,
                        stop=True,
                    )
                    rb = nrm.tile([DH, 512], F32, tag="rb", bufs=2, name=f"rb{uid}")
                    # ~51-ULP approx is ample for softmax denominators and ~5x
                    # faster than the exact multi-pass InstReciprocal
                    nc.vector.reciprocal_approx_fast(out=rb, in_=bc[0:DH, :])
                    nc.vector.tensor_tensor(
                        out=aoT[j][po : po + DH, cs : cs + 512],
                        in0=ao_sb[0:DH, :],
                        in1=rb,
                        op=ALU.mult,
                    )

                return norm

            def attn_v(i):
                qb, pj, kb = items[i]
                q0 = qb * 512
                if kb == 0:
                    ao_tiles[(qb, pj)] = (
                        aop.tile([DH + 1, 512], F32, tag="ao", name=f"aoA{qb}_{pj}"),
                        aop.tile([DH + 1, 512], F32, tag="ao", name=f"aoB{qb}_{pj}"),
                    )
                halves = ao_tiles[(qb, pj)]
                ex = ex_tiles.pop(i)
                for c in range(2):
                    nc.tensor.matmul(
                        halves[c],
                        lhsT=(v_sb[:, kb, 2 * pj + c, :]),
                        rhs=(ex[:, c * 512 : (c + 1) * 512]),
                        start=(kb == 0),
                        stop=(kb == NT - 1),
                    )
                if kb == NT - 1:
                    # evict the unnormalized accumulator halves; normalize is
                    # deferred so the PE broadcast never waits on these copies
                    for c in range(2):
                        # bf16 eviction: aoT is bf16 anyway, and the bf16
                        # denominator row makes the broadcast matmul stream at
                        # full rate (~240ns vs 570ns fp32)
                        ao_sb = nrm.tile(
                            [DH + 1, 512], BF16, tag="ao_sb", name=f"aosb{i}_{c}"
                        )
                        nc.vector.tensor_copy(ao_sb, halves[c])
                        # spread the two PE broadcasts well apart: each eats
                        # ~500ns of PE time and the steady-state slack is only
                        # ~100ns/item -- bunched they stall the exp stream
                        pending.append(
                            (i + 3 + 3 * c, make_norm(pj, c * DH, q0, ao_sb, f"{i}_{c}"))
                        )

            def outproj_tile(mt, pool, tag, evict_act=False, wide=False):
                # wide: borrow a [P,1024] tile from the (tail-idle) score ring
                # and target its first bank, so consecutive tail tiles double-
                # buffer instead of serializing on the single o-pool buffer
                if wide:
                    ps = pool.tile([P, 1024], F32, tag=tag, name=f"o{mt}")[:, 0:D]
                else:
                    ps = pool.tile([P, D], F32, tag=tag, name=f"o{mt}")
                for kt in range(2):
                    nc.tensor.matmul(
                        ps,
                        lhsT=(aoT[kt][:, mt * P : (mt + 1) * P]),
                        rhs=(w_o_sb[:, kt, :]),
                        start=(kt == 0),
                        stop=(kt == 1),
                    )
                ot = osb.tile([P, D], F32, tag="ot", name=f"ot{mt}")
                if evict_act:
                    # tail only: ACT is idle there, DVE is busy with norms
                    nc.scalar.activation(out=ot, in_=ps, func=AFT.Copy)
                else:
                    nc.vector.tensor_copy(ot, ps)
                nc.sync.dma_start(out=out[mt * P : (mt + 1) * P, :], in_=ot)

            # q-block qb (tokens 512qb..512qb+511 = mt 4qb..4qb+3) is fully
            # normalized ~3 items after its second unit ends at 32qb+31;
            # spread its output-projection tiles one per item from 32qb+36
            # q-block qb is fully normalized once (qb,P1)'s R-norm lands at
            # item 32qb+37; outprojs start at +38 (emitting one EARLIER would
            # wedge the in-order PE behind a norm that is emitted later)
            inject = {}
            for qb in range(NQ - 1):
                for k in range(4):
                    inject[32 * qb + 38 + 3 * k] = 4 * qb + k

            DEPTH = 2
            for i in range(min(DEPTH, len(items))):
                sc_exp(i)
            for i in range(len(items)):
                attn_v(i)
                if i + DEPTH < len(items):
                    sc_exp(i + DEPTH)
                if pending and pending[0][0] <= i:
                    pending.pop(0)[1]()
                if i in inject:
                    outproj_tile(inject[i], opp, "o")
            for _, fn in pending:
                fn()
            # keep the PE array (and HAM clock) warm while DVE finishes the
            # last unit's normalize chain, so the final output projections run
            # at 2.4 GHz instead of re-throttled 1.2
            wt = scp.tile([P, 1024], F32, tag="sc", name="wtail")
            for _ in range(12):
                nc.tensor.matmul(
                    wt[:, 0:512], lhsT=identity, rhs=warm_sb, start=True, stop=True
                )
            for idx, mt in enumerate(range(4 * (NQ - 1), NT)):
                if idx % 2 == 0:
                    outproj_tile(mt, opp, "o", evict_act=True)
                else:
                    outproj_tile(mt, scp, "sc", evict_act=True, wide=True)

    nc.compile()
    return nc


_NC_CACHE = None
_LAST_RESULT = None


def kernel(x, ln_scale, ln_bias, w_qkv, w_out):
    global _NC_CACHE, _LAST_RESULT
    if _NC_CACHE is None:
        _NC_CACHE = build_kernel()
    nc = _NC_CACHE

    import ml_dtypes

    x = np.asarray(x, np.float32)
    w_eff = (np.asarray(ln_scale, np.float32)[:, None] * np.asarray(w_qkv, np.float32))
    b_row = np.asarray(ln_bias, np.float32) @ np.asarray(w_qkv, np.float32)
    w_eff = w_eff.astype(ml_dtypes.bfloat16)
    w_out = np.asarray(w_out, np.float32).astype(ml_dtypes.bfloat16)

    in_maps = []
    for c in range(8):
        b, g = c // 2, c % 2
        s = slice(FPC * g, FPC * g + FPC)
        ks = slice(512 + FPC * g, 512 + FPC * g + FPC)
        vs = slice(1024 + FPC * g, 1024 + FPC * g + FPC)
        in_maps.append(
            {
                "xb": np.ascontiguousarray(x[b]),
                "wq": np.ascontiguousarray(w_eff[:, s]),
                "wk": np.ascontiguousarray(w_eff[:, ks]),
                "wv": np.ascontiguousarray(w_eff[:, vs]),
                "wo": np.ascontiguousarray(w_out[s, :]),
                "bq": np.ascontiguousarray(b_row[s]),
                "bk": np.ascontiguousarray(b_row[ks]),
                "bv": np.ascontiguousarray(b_row[vs]),
            }
        )
    res = run_bass_kernel_spmd(nc, in_maps, core_ids=list(range(8)))
    _LAST_RESULT = res
    outs = [res.results[c]["out"] for c in range(8)]
    return np.stack([outs[2 * b] + outs[2 * b + 1] for b in range(B)]).astype(
        np.float32
    )


if __name__ == "__main__":
    xs = np.random.randn(B, N, D).astype(np.float32)
    o = kernel(
        x=xs,
        ln_scale=np.ones(D, np.float32),
        ln_bias=np.zeros(D, np.float32),
        w_qkv=(np.random.randn(D, 3 * H * DH) / np.sqrt(D)).astype(np.float32),
        w_out=(np.random.randn(H * DH, D) / np.sqrt(H * DH)).astype(np.float32),
    )
    print(o.shape, o.dtype)


# revision 51
# speedup vs baseline: 1.1783x; 1.1690x over previous
"""Trainium2 Bass kernel for pre-LN multi-head self-attention.

Module: y = LN(x); qkv = y @ w_qkv; attention(8 heads, dh=64); out = ao @ w_out
Shapes: x [4, 2048, 512], w_qkv [512, 1536], w_out [512, 512], fp32.

Sharding (8 cores): core c -> batch b = c//2, head-group g = c%2 (4 heads).
Each core computes LN + QKV (its head slice) + attention + a partial output
projection (its heads' rows of w_out); the host sums the two partials per batch.

Per-core dataflow (transpose-free except one PE transpose of y):
  LN in natural [tok, d] layout (bn_stats) -> PE-transpose y -> yT [d, tok]
  Q^T, K^T = w^T @ yT   (features on partitions -- natural lhsT layout)
  V natural [tok, feat] with a fused ones-column so attn@V also accumulates
  the softmax denominator (row 64 of the PSUM accumulator).
  scoresT [k, q] = K^T.T @ Q^T per 128-k-token block.  The two heads of a
  head-pair sit at partitions 0-63 / 64-127 of qT/kT (PE row groups 0/64),
  so their K=64 score matmuls are emitted as a row-tiled pair that runs
  CONCURRENTLY on the array: one item = (512-q-block, head-pair, k-block),
  its two score halves landing in one [128,1024] PSUM tile, exp'd by a
  single ACT instruction.  attn@V accumulates per head over k-blocks in
  PSUM (two independent 1-bank accumulators); per-head 1/sumexp is
  broadcast across the 64 dh partitions with a K=1 PE matmul against a
  ones-row (no DRAM roundtrip), then reciprocal+mult on DVE; the output
  projection consumes aoT directly as lhsT.
ln_scale/ln_bias are folded into w_qkv on the host (w_eff = scale*W,
bias_row = bias@W added per-feature on device), so the device LN is pure
normalize.  Matmul operands are bf16 (PSUM accumulation stays fp32).
Stage D runs a depth-2 software pipeline with attn@V(i) issued BEFORE
scores(i+2) so the in-order PE never stalls on a score-PSUM bank held by
a pending exp; row-tiled scores + full-K attn@V keep the array active
enough for the HAM clock gate to hold K=8/8 (2.4 GHz).  Normalize work
for a finished unit is deferred two items so its PE broadcast never waits
on the DVE eviction copy.  A dummy exp after stage A hoists the ACT
exp-table load out of the attention phase.
"""

import sys

if "/opt/trn_rl_repo" not in sys.path:
    sys.path.insert(0, "/opt/trn_rl_repo")

from contextlib import ExitStack

import numpy as np

import concourse.bass as bass
import concourse.tile as tile
from concourse.masks import make_identity
from concourse import bacc, mybir
from concourse.bass_utils import run_bass_kernel_spmd

B, N, D = 4, 2048, 512
H, DH = 8, 64
HPC = 4                 # heads per core
FPC = HPC * DH          # 256 features per core
P = 128
NT = N // P             # 16 token tiles
DT = D // P             # 4 d tiles
NQ = N // 512           # 4 q-blocks of 512
EPS = 1e-6
SCALE = DH ** -0.5
F32 = mybir.dt.float32
F32R = mybir.dt.float32r
BF16 = mybir.dt.bfloat16
ALU = mybir.AluOpType
AFT = mybir.ActivationFunctionType




def build_kernel():
    nc = bacc.Bacc("TRN2", target_bir_lowering=False, debug=False)
    xb = nc.dram_tensor("xb", [N, D], F32, kind="ExternalInput").ap()
    wq = nc.dram_tensor("wq", [D, FPC], BF16, kind="ExternalInput").ap()
    wk = nc.dram_tensor("wk", [D, FPC], BF16, kind="ExternalInput").ap()
    wv = nc.dram_tensor("wv", [D, FPC], BF16, kind="ExternalInput").ap()
    wo = nc.dram_tensor("wo", [FPC, D], BF16, kind="ExternalInput").ap()
    bq = nc.dram_tensor("bq", [FPC], F32, kind="ExternalInput").ap()
    bk = nc.dram_tensor("bk", [FPC], F32, kind="ExternalInput").ap()
    bv = nc.dram_tensor("bv", [FPC], F32, kind="ExternalInput").ap()
    out = nc.dram_tensor("out", [N, D], F32, kind="ExternalOutput").ap()

    with tile.TileContext(nc, pool_alloc_mode="queue") as tc, ExitStack() as ctx:
        consts = ctx.enter_context(tc.tile_pool(name="consts", bufs=1))

        big = ctx.enter_context(tc.tile_pool(name="big", bufs=1))

        identity = consts.tile([P, P], BF16)
        make_identity(nc, identity)
        eps_t = consts.tile([P, 1], F32)
        nc.vector.memset(eps_t, EPS)
        # ones row living at PARTITION 64: lhsT for the K=1 denominator
        # broadcast must share its base partition with the rhs (the denom row
        # sits at partition 64 of the attn@V accumulator)
        ones_m = consts.tile([DH + 1, DH], BF16)
        nc.vector.memset(ones_m, 1.0)
        ones_row = ones_m[DH : DH + 1, :]
        exp_warm = consts.tile([P, 1], F32)
        warm_sb = consts.tile([P, 512], BF16)
        nc.vector.memset(warm_sb, 0.0)

        yT = [big.tile([P, N], BF16, tag=f"yT{j}", name=f"yT{j}") for j in range(DT)]
        qT = [big.tile([P, N], BF16, tag=f"qT{j}", name=f"qT{j}") for j in range(2)]
        kT = [big.tile([P, N], BF16, tag=f"kT{j}", name=f"kT{j}") for j in range(2)]
        aoT = [big.tile([P, N], BF16, tag=f"aoT{j}", name=f"aoT{j}") for j in range(2)]
        v_sb = big.tile([P, NT, HPC, DH + 1], BF16)
        ones_col = consts.tile([P, 1], F32)
        nc.vector.memset(ones_col, 1.0)
        nc.vector.tensor_copy(
            v_sb[:, :, :, DH : DH + 1],
            ones_col[:, 0:1].to_broadcast((P, NT, HPC, 1)),
        )

        # weights: [d, f] -> sbuf [p, dt, f] -- issued before the LN phase so
        # the transfers overlap it and QKV chunks can start with token-group 0
        w_k_sb = consts.tile([P, DT, FPC], BF16)
        nc.sync.dma_start(out=w_k_sb, in_=wk.rearrange("(t p) f -> p t f", p=P))
        w_q_sb = consts.tile([P, DT, FPC], BF16)
        nc.sync.dma_start(out=w_q_sb, in_=wq.rearrange("(t p) f -> p t f", p=P))
        w_v_sb = consts.tile([P, DT, FPC], BF16)
        nc.sync.dma_start(out=w_v_sb, in_=wv.rearrange("(t p) f -> p t f", p=P))
        w_o_sb = consts.tile([P, 2, D], BF16)
        nc.sync.dma_start(out=w_o_sb, in_=wo.rearrange("(t p) f -> p t f", p=P))
        bq_sb = consts.tile([P, 2], F32)
        nc.sync.dma_start(out=bq_sb, in_=bq.rearrange("(t p) -> p t", p=P))
        bk_sb = consts.tile([P, 2], F32)
        nc.sync.dma_start(out=bk_sb, in_=bk.rearrange("(t p) -> p t", p=P))
        bv_b = consts.tile([P, FPC], F32)
        bv_bcast = bass.AP(tensor=bv.tensor, offset=bv.offset, ap=[[0, P]] + list(bv.ap))
        nc.sync.dma_start(out=bv_b, in_=bv_bcast)

        # ---- Stages A-C interleaved per 4-tile token group: LayerNorm,
        # transpose y -> yT, then the QKV chunks for just that group's token
        # columns.  The PE's QKV matmuls overlap the next group's LN (DVE) ----
        with tc.tile_pool(name="ln", bufs=3) as ln, tc.tile_pool(
            name="tp_psum", bufs=2, space="PSUM"
        ) as tpp, tc.tile_pool(
            name="c_psum", bufs=2, space="PSUM"
        ) as cpp, tc.tile_pool(name="v_psum", bufs=2, space="PSUM") as vpp:
            # full-width dummy matmuls fill the PE-idle LayerNorm window: the
            # HAM clock gate un-throttles ~10us in, so the real transposes and
            # QKV chunks run at 2.4 GHz instead of warming up mid-prelude.
            # 36 covers the window until group-0's transposes (~18us) -- any
            # >3.4us PE-idle stretch re-throttles the clock
            wp = tpp.tile([P, 512], F32, tag="warm", bufs=1)
            for _ in range(36):
                nc.tensor.matmul(wp, lhsT=identity, rhs=warm_sb, start=True, stop=True)
            for ig in range(NT // 4):  # groups of 4 token tiles
                y_ts = []
                for ii in range(4):
                    i = ig * 4 + ii
                    x_t = ln.tile([P, D], F32, tag="x", bufs=5)
                    nc.sync.dma_start(out=x_t, in_=xb[i * P : (i + 1) * P, :])
                    stats = ln.tile([P, 6], F32, tag="stats")
                    nc.vector.bn_stats(out=stats, in_=x_t)
                    mv = ln.tile([P, 2], F32, tag="mv")
                    nc.vector.bn_aggr(out=mv, in_=stats)
                    std = ln.tile([P, 1], F32, tag="std")
                    nc.scalar.activation(
                        out=std, in_=mv[:, 1:2], func=AFT.Sqrt, bias=eps_t[:, 0:1]
                    )
                    rstd = ln.tile([P, 1], F32, tag="rstd")
                    nc.vector.reciprocal(out=rstd, in_=std)
                    if i == NT - 1:
                        # depends on the final Sqrt: forces the ACT exp table
                        # set to load AFTER the sqrt set, so it is resident
                        # for stage D (the scheduler reorders by deps, not
                        # program order)
                        nc.scalar.activation(out=exp_warm, in_=std, func=AFT.Exp)
                    y_t = ln.tile([P, D], BF16, tag="y", bufs=6)
                    nc.vector.tensor_scalar(
                        out=y_t,
                        in0=x_t,
                        scalar1=mv[:, 0:1],
                        scalar2=rstd[:, 0:1],
                        op0=ALU.subtract,
                        op1=ALU.mult,
                    )
                    y_ts.append(y_t)
                for j in range(DT):
                    pt = tpp.tile([P, 512], BF16, tag="tp")
                    for ii in range(4):
                        nc.tensor.transpose(
                            pt[:, ii * P : (ii + 1) * P],
                            y_ts[ii][:, j * P : (j + 1) * P],
                            identity,
                        )
                    nc.scalar.activation(
                        out=yT[j][:, ig * 512 : (ig + 1) * 512],
                        in_=pt,
                        func=AFT.Copy,
                    )
                if ig == 0:
                    # bridge the group-0 transpose window (transpose-mode does
                    # not register as PE-busy for the HAM activity monitor)
                    for _ in range(6):
                        nc.tensor.matmul(
                            wp, lhsT=identity, rhs=warm_sb, start=True, stop=True
                        )
                g0 = ig * 512
                for wi, (w_sb, b_sb, dstT) in enumerate(
                    ((w_k_sb, bk_sb, kT), (w_q_sb, bq_sb, qT))
                ):
                    for j in range(2):
                        ps = cpp.tile([P, 512], F32, tag="qk", name=f"qk{wi}{j}_{ig}")
                        for dt in range(DT):
                            nc.tensor.matmul(
                                ps,
                                lhsT=(w_sb[:, dt, j * P : (j + 1) * P]),
                                rhs=(yT[dt][:, g0 : g0 + 512]),
                                start=(dt == 0),
                                stop=(dt == DT - 1),
                            )
                        # bias-add evictions alternate ACT/DVE to balance the
                        # two psum-capable engines
                        if (wi + j) % 2 == 0:
                            nc.scalar.activation(
                                out=dstT[j][:, g0 : g0 + 512],
                                in_=ps,
                                func=AFT.Identity,
                                bias=b_sb[:, j : j + 1],
                            )
                        else:
                            nc.vector.tensor_scalar(
                                out=dstT[j][:, g0 : g0 + 512],
                                in0=ps,
                                scalar1=b_sb[:, j : j + 1],
                                scalar2=None,
                                op0=ALU.add,
                            )
                for i in range(ig * 4, ig * 4 + 4):
                    ps = vpp.tile([P, FPC], F32, tag="v", name=f"v{i}")
                    for dt in range(DT):
                        nc.tensor.matmul(
                            ps,
                            lhsT=(yT[dt][:, i * P : (i + 1) * P]),
                            rhs=(w_v_sb[:, dt, :]),
                            start=(dt == 0),
                            stop=(dt == DT - 1),
                        )
                    nc.vector.tensor_tensor(
                        out=v_sb[:, i, :, 0:DH],
                        in0=ps.rearrange("p (h d) -> p h d", h=HPC),
                        in1=bv_b.rearrange("p (h d) -> p h d", h=HPC),
                        op=ALU.add,
                    )

        # ---- Stage D: attention, units of (512-q-block, head-pair) ----
        # The two heads of a pair live at partitions 0-63 / 64-127 of qT/kT,
        # i.e. PE row groups 0 and 64: their K=64 score matmuls are emitted as
        # a row-tiled pair (tile_position (0,0) / (64,0)) and run CONCURRENTLY
        # on the array -- full 128-row activity per score step, 2x throughput,
        # and dense enough for the HAM clock gate to hold K=8/8.
        with tc.tile_pool(name="sc_psum", bufs=2, space="PSUM") as scp, tc.tile_pool(
            name="ao_psum", bufs=3, space="PSUM"
        ) as aop, tc.tile_pool(
            name="o_psum", bufs=1, space="PSUM"
        ) as opp, tc.tile_pool(name="exp_sb", bufs=6) as exps, tc.tile_pool(
            name="nrm", bufs=4
        ) as nrm, tc.tile_pool(name="o_sb", bufs=3) as osb:
            items = [
                (qb, pj, kb) for qb in range(NQ) for pj in range(2) for kb in range(NT)
            ]
            ex_tiles = {}
            ao_tiles = {}
            pending = []  # (ready_at_item, emit_closure) for unit normalizes

            def sc_exp(i):
                qb, pj, kb = items[i]
                q0 = qb * 512
                sc = scp.tile([P, 1024], F32, tag="sc", name=f"sc{i}")
                for c in range(2):
                    po = c * DH
                    nc.tensor.matmul(
                        sc[:, c * 512 : (c + 1) * 512],
                        lhsT=(kT[pj][po : po + DH, kb * P : (kb + 1) * P]),
                        rhs=(qT[pj][po : po + DH, q0 : q0 + 512]),
                        start=True,
                        stop=True,
                    )
                ex = exps.tile([P, 1024], BF16, tag="ex", name=f"ex{i}")
                nc.scalar.activation(out=ex, in_=sc, func=AFT.Exp, scale=SCALE)
                ex_tiles[i] = ex

            def make_norm(j, po, cs, ao_sb, uid):
                def norm():
                    # broadcast the denominator row across the dh partitions
                    # with a K=1 matmul (PSUM tile shared with the outproj
                    # ring -- usage windows never overlap), then normalize
                    bc = opp.tile([P, D], F32, tag="o", name=f"bc{uid}")
                    nc.tensor.matmul(
                        bc[0:DH, :],
                        lhsT=ones_row,
                        rhs=ao_sb[DH : DH + 1, :],
                        start=True,
                        stop=True,
                    )
                    rb = nrm.tile([DH, 512], F32, tag="rb", bufs=2, name=f"rb{uid}")
                    # ~51-ULP approx is ample for softmax denominators and ~5x
                    # faster than the exact multi-pass InstReciprocal
                    nc.vector.reciprocal_approx_fast(out=rb, in_=bc[0:DH, :])
                    nc.vector.tensor_tensor(
                        out=aoT[j][po : po + DH, cs : cs + 512],
                        in0=ao_sb[0:DH, :],
                        in1=rb,
                        op=ALU.mult,
                    )

                return norm

            def attn_v(i):
                qb, pj, kb = items[i]
                q0 = qb * 512
                if kb == 0:
                    ao_tiles[(qb, pj)] = (
                        aop.tile([DH + 1, 512], F32, tag="ao", name=f"aoA{qb}_{pj}"),
                        aop.tile([DH + 1, 512], F32, tag="ao", name=f"aoB{qb}_{pj}"),
                    )
                halves = ao_tiles[(qb, pj)]
                ex = ex_tiles.pop(i)
                for c in range(2):
                    nc.tensor.matmul(
                        halves[c],
                        lhsT=(v_sb[:, kb, 2 * pj + c, :]),
                        rhs=(ex[:, c * 512 : (c + 1) * 512]),
                        start=(kb == 0),
                        stop=(kb == NT - 1),
                    )
                if kb == NT - 1:
                    # evict the unnormalized accumulator halves; normalize is
                    # deferred so the PE broadcast never waits on these copies
                    for c in range(2):
                        # bf16 eviction: aoT is bf16 anyway, and the bf16
                        # denominator row makes the broadcast matmul stream at
                        # full rate (~240ns vs 570ns fp32)
                        ao_sb = nrm.tile(
                            [DH + 1, 512], BF16, tag="ao_sb", name=f"aosb{i}_{c}"
                        )
                        nc.vector.tensor_copy(ao_sb, halves[c])
                        # spread the two PE broadcasts well apart: each eats
                        # ~500ns of PE time and the steady-state slack is only
                        # ~100ns/item -- bunched they stall the exp stream
                        pending.append(
                            (i + 3 + 3 * c, make_norm(pj, c * DH, q0, ao_sb, f"{i}_{c}"))
                        )

            def outproj_tile(mt, pool, tag, evict_act=False):
                ps = pool.tile([P, D], F32, tag=tag, name=f"o{mt}")
                for kt in range(2):
                    nc.tensor.matmul(
                        ps,
                        lhsT=(aoT[kt][:, mt * P : (mt + 1) * P]),
                        rhs=(w_o_sb[:, kt, :]),
                        start=(kt == 0),
                        stop=(kt == 1),
                    )
                ot = osb.tile([P, D], F32, tag="ot", name=f"ot{mt}")
                if evict_act:
                    # tail only: ACT is idle there, DVE is busy with norms
                    nc.scalar.activation(out=ot, in_=ps, func=AFT.Copy)
                else:
                    nc.vector.tensor_copy(ot, ps)
                nc.sync.dma_start(out=out[mt * P : (mt + 1) * P, :], in_=ot)

            # q-block qb (tokens 512qb..512qb+511 = mt 4qb..4qb+3) is fully
            # normalized ~3 items after its second unit ends at 32qb+31;
            # spread its output-projection tiles one per item from 32qb+36
            # q-block qb is fully normalized once (qb,P1)'s R-norm lands at
            # item 32qb+37; outprojs start at +38 (emitting one EARLIER would
            # wedge the in-order PE behind a norm that is emitted later)
            inject = {}
            for qb in range(NQ - 1):
                for k in range(4):
                    inject[32 * qb + 38 + 3 * k] = 4 * qb + k

            DEPTH = 2
            for i in range(min(DEPTH, len(items))):
                sc_exp(i)
            for i in range(len(items)):
                attn_v(i)
                if i + DEPTH < len(items):
                    sc_exp(i + DEPTH)
                if pending and pending[0][0] <= i:
                    pending.pop(0)[1]()
                if i in inject:
                    outproj_tile(inject[i], opp, "o")
            for _, fn in pending:
                fn()
            # keep the PE array (and HAM clock) warm while DVE finishes the
            # last unit's normalize chain, so the final output projections run
            # at 2.4 GHz instead of re-throttled 1.2
            wt = scp.tile([P, 1024], F32, tag="sc", name="wtail")
            for _ in range(12):
                nc.tensor.matmul(
                    wt[:, 0:512], lhsT=identity, rhs=warm_sb, start=True, stop=True
                )
            for mt in range(4 * (NQ - 1), NT):
                outproj_tile(mt, opp, "o", evict_act=True)

    nc.compile()
    return nc


_NC_CACHE = None
_LAST_RESULT = None


def kernel(x, ln_scale, ln_bias, w_qkv, w_out):
    global _NC_CACHE, _LAST_RESULT
    if _NC_CACHE is None:
        _NC_CACHE = build_kernel()
    nc = _NC_CACHE

    import ml_dtypes

    x = np.asarray(x, np.float32)
    w_eff = (np.asarray(ln_scale, np.float32)[:, None] * np.asarray(w_qkv, np.float32))
    b_row = np.asarray(ln_bias, np.float32) @ np.asarray(w_qkv, np.float32)
    w_eff = w_eff.astype(ml_dtypes.bfloat16)
    w_out = np.asarray(w_out, np.float32).astype(ml_dtypes.bfloat16)

    in_maps = []
    for c in range(8):
        b, g = c // 2, c % 2
        s = slice(FPC * g, FPC * g + FPC)
        ks = slice(512 + FPC * g, 512 + FPC * g + FPC)
        vs = slice(1024 + FPC * g, 1024 + FPC * g + FPC)
        in_maps.append(
            {
                "xb": np.ascontiguousarray(x[b]),
                "wq": np.ascontiguousarray(w_eff[:, s]),
                "wk": np.ascontiguousarray(w_eff[:, ks]),
                "wv": np.ascontiguousarray(w_eff[:, vs]),
                "wo": np.ascontiguousarray(w_out[s, :]),
                "bq": np.ascontiguousarray(b_row[s]),
                "bk": np.ascontiguousarray(b_row[ks]),
                "bv": np.ascontiguousarray(b_row[vs]),
            }
        )
    res = run_bass_kernel_spmd(nc, in_maps, core_ids=list(range(8)))
    _LAST_RESULT = res
    outs = [res.results[c]["out"] for c in range(8)]
    return np.stack([outs[2 * b] + outs[2 * b + 1] for b in range(B)]).astype(
        np.float32
    )


if __name__ == "__main__":
    xs = np.random.randn(B, N, D).astype(np.float32)
    o = kernel(
        x=xs,
        ln_scale=np.ones(D, np.float32),
        ln_bias=np.zeros(D, np.float32),
        w_qkv=(np.random.randn(D, 3 * H * DH) / np.sqrt(D)).astype(np.float32),
        w_out=(np.random.randn(H * DH, D) / np.sqrt(H * DH)).astype(np.float32),
    )
    print(o.shape, o.dtype)
